# revision 17
# baseline (speedup 1.0000x reference)
"""Trainium2 Bass kernel for nn_CrossLocal (cross-attention + convs + BN +
bilinear resizes), distributed over 8 NeuronCores in a SINGLE fused launch.

Sharding: core = 2*b + qh owns half the query rows of batch b's 64x64 cross
image (32 rows = 2048 queries).  For qh=1 the row axis is mirrored on the
host so every core runs the identical program ("outer edge" at local row 0,
cross-core halo after local row 31).

Host precomputes (tiny, <<1ms of matmul): y = theta_w@cr, x = g_w@cr,
r[m] = g_b.y_m (softmax-invariant bias), z = phi(avgpool2x2(main)) with an
extra ones column that accumulates the softmax denominator for free.

Device pipeline per core:
  S_T[m-tile, q] = y_tile^T @ x  (K=32 bf16 matmuls, 4x PE row-tiling)
  es = exp(S + r): even m-tiles on ScalarE (table exp), odd m-tiles on DVE
       via the Schraudolph bit-trick (round(a*S + b) as int16 = bf16 bits
       of 2^(S+r)/log-scale; softmax averaging washes out the ~3.5% error)
  num/den = z_aug^T @ es accumulated in PSUM (K=128 bf16)
  h = (W@num + b*den)/den via ones-broadcast + reciprocal
  BN stats: two pipelined AllGathers (half A carries the halo edge row so
  the collective overlaps the second half's compute), affine folded into
  the 2x bilinear upsample output, + residual, fp16 I/O throughout.
"""

import os
from contextlib import ExitStack

import numpy as np
import ml_dtypes

import concourse.bass as bass
import concourse.tile as tile
from concourse import bacc, mybir
from concourse.bass_utils import run_bass_kernel_spmd

f32 = mybir.dt.float32
f32r = mybir.dt.float32r
fp16 = mybir.dt.float16
bf16 = mybir.dt.bfloat16
i16 = mybir.dt.int16
AF = mybir.ActivationFunctionType
ALU = mybir.AluOpType

NPBF16 = ml_dtypes.bfloat16

B, C, CI = 4, 64, 32
HM, HC = 128, 64
NC = HC * HC          # 4096 keys
NQ = NC // 2          # 2048 queries per core
NT = NC // 128        # 32 m-tiles
BN_EPS = 1e-5
NSTAT = float(B * NC)
CORES = list(range(8))

# Schraudolph 2^x constants for bf16 bit construction (7 mantissa bits):
# bits = round(A*(S + r) + CC);  A = 128*log2(e), CC = 16256 - 128*delta*
A_EXP = 128 * 1.4426950408889634
C_EXP = 16251.4432

_cache = {}
last_profile = {}


def _run(nc, in_maps, name):
    trace = os.environ.get("BASS_PROFILE", "") == "1"
    tmpdir = None
    if trace:
        tmpdir = os.path.join("/tmp/bass_traces", name)
        os.makedirs(tmpdir, exist_ok=True)
    br = run_bass_kernel_spmd(
        nc, in_maps, core_ids=CORES, trace=trace, tmpdir=tmpdir
    )
    if trace:
        last_profile[name] = br
    return br.results


def _build():
    nc = bacc.Bacc("TRN2", target_bir_lowering=False, debug=False,
                   num_devices=len(CORES))
    d_y4 = nc.dram_tensor("y4", [128, NC], bf16, kind="ExternalInput").ap()
    d_x4 = nc.dram_tensor("x4", [128, NQ], bf16, kind="ExternalInput").ap()
    d_z = nc.dram_tensor("zaug", [128, NT * 33], bf16, kind="ExternalInput").ap()
    d_wwt = nc.dram_tensor("wwt", [33, C], f32, kind="ExternalInput").ap()
    d_ones = nc.dram_tensor("ones64", [1, C], f32, kind="ExternalInput").ap()
    d_mask = nc.dram_tensor("mask", [C, 8 * 64], f32, kind="ExternalInput").ap()
    d_mn = nc.dram_tensor("mainr", [128, 32 * HM], fp16, kind="ExternalInput").ap()
    d_g16 = nc.dram_tensor("g16", [C, 1], f32, kind="ExternalInput").ap()
    d_gam = nc.dram_tensor("gam", [C, 1], f32, kind="ExternalInput").ap()
    d_bet = nc.dram_tensor("bet", [C, 1], f32, kind="ExternalInput").ap()
    d_o = nc.dram_tensor("outp", [128, 32 * HM], fp16, kind="ExternalOutput").ap()

    with ExitStack() as ctx:
        tc = ctx.enter_context(tile.TileContext(nc))
        const = ctx.enter_context(tc.tile_pool(name="const", bufs=1))
        work = ctx.enter_context(tc.tile_pool(name="work", bufs=1))
        half = ctx.enter_context(tc.tile_pool(name="half", bufs=2))
        es_s = ctx.enter_context(tc.tile_pool(name="es_s", bufs=6))
        ps_s = ctx.enter_context(tc.tile_pool(name="ps_s", bufs=3, space="PSUM"))
        ps_acc = ctx.enter_context(tc.tile_pool(name="ps_acc", bufs=1, space="PSUM"))
        dram = ctx.enter_context(tc.tile_pool(name="dram", bufs=1, space="DRAM"))

        # --- warm the exp table ASAP ---
        warm = const.tile([1, 1], f32, tag="warm")
        nc.vector.memset(warm, 0.0)
        nc.scalar.activation(out=warm, in_=warm, func=AF.Exp)

        # --- constant loads ---
        y4 = const.tile([128, NC], bf16, tag="y4")
        nc.sync.dma_start(out=y4, in_=d_y4)
        x4 = const.tile([128, NQ], bf16, tag="x4")
        nc.sync.dma_start(out=x4, in_=d_x4)
        z_sb = const.tile([128, NT, 33], bf16, tag="z_sb")
        nc.sync.dma_start(out=z_sb, in_=d_z.rearrange("p (t s) -> p t s", s=33))
        wwt = const.tile([33, C], f32r, tag="wwt")
        nc.gpsimd.dma_start(out=wwt, in_=d_wwt)
        ones64 = const.tile([1, C], f32r, tag="ones64")
        nc.gpsimd.dma_start(out=ones64, in_=d_ones)
        mask = const.tile([C, 8, 64], f32, tag="mask")
        nc.sync.dma_start(out=mask, in_=d_mask.rearrange("c (r w) -> c r w", w=64))
        mainr = const.tile([128, 32 * HM], fp16, tag="mainr")
        nc.gpsimd.dma_start(out=mainr, in_=d_mn)
        g16 = const.tile([C, 1], f32, tag="g16")
        nc.sync.dma_start(out=g16, in_=d_g16)
        gam = const.tile([C, 1], f32, tag="gam")
        nc.sync.dma_start(out=gam, in_=d_gam)
        bet = const.tile([C, 1], f32, tag="bet")
        nc.sync.dma_start(out=bet, in_=d_bet)

        # --- persistent work tiles ---
        h = work.tile([C, NQ], fp16, tag="h")           # local q order
        junk = work.tile([C, 1024], fp16, tag="junk")   # gpsimd stt scratch
        sh = work.tile([C, 2], f32, tag="sh")           # per-half sum(h)
        sq = work.tile([C, 2], f32, tag="sq")           # per-half sum(h^2)
        acc_all = ps_acc.tile([128, 1024], f32, tag="acc")

        # DRAM bounce buffers for the two AllGathers
        ag1_in = dram.tile([C, 68], f32)
        ag1_out = dram.tile([8 * C, 68], f32)
        ag2_in = dram.tile([C, 2], f32)
        ag2_out = dram.tile([8 * C, 2], f32)

        # attention: halves are processed edge-rows-first (half A = local
        # rows 16..31 = cols 1024:2048) so the halo row ships with AR#1.
        def emit_az(hh, t, es_bf):
            accp = acc_all[64 * hh:64 * hh + 33, :]
            for c in range(2):
                nc.tensor.matmul(
                    out=accp[:, 512 * c:512 * c + 512],
                    lhsT=z_sb[:, t, :],
                    rhs=es_bf[:, 512 * c:512 * c + 512],
                    start=(t == 0), stop=(t == NT - 1),
                    skip_group_check=True,
                    tile_position=(0, 64 * hh),
                )

        batches = [list(range(b, min(b + 3, NT))) for b in range(0, NT, 3)]
        for hh, qoff in ((0, 1024), (1, 0)):
            pend = []
            for bts in batches:
                sps = {}
                for t in bts:
                    j = t % 4
                    s_ps = ps_s.tile([128, 1024], f32, tag="s")
                    sps[t] = s_ps
                    for k in range(2):
                        nc.tensor.matmul(
                            out=s_ps[:, 512 * k:512 * k + 512],
                            lhsT=y4[32 * j:32 * j + 32, 128 * t:128 * t + 128],
                            rhs=x4[32 * j:32 * j + 32,
                                   qoff + 512 * k:qoff + 512 * k + 512],
                            start=True, stop=True,
                            skip_group_check=True,
                            tile_position=(32 * j, 0),
                        )
                for t in bts:
                    es = es_s.tile([128, 1024], bf16, tag="es")
                    nc.scalar.activation(out=es, in_=sps[t], func=AF.Exp)
                    pend.append((t, es))
                # drain attn@z one batch behind so the PE never waits on
                # an exp that was just issued
                while len(pend) > len(bts):
                    td, esd = pend.pop(0)
                    emit_az(hh, td, esd)
            for (td, esd) in pend:
                emit_az(hh, td, esd)

            # --- epilogue for this half ---
            accp = acc_all[64 * hh:64 * hh + 33, :]
            nsb = half.tile([33, 1024], f32r, tag="nsb")
            nc.vector.tensor_copy(out=nsb, in_=accp)
            for c in range(2):
                sl = slice(512 * c, 512 * c + 512)
                dbc = acc_all[0:C, 0:512]
                nc.tensor.matmul(
                    out=dbc, lhsT=ones64, rhs=nsb[0:1, sl],
                    start=True, stop=True, skip_group_check=True,
                )
                rec = half.tile([C, 512], f32, tag="rec")
                nc.vector.reciprocal_approx_fast(out=rec, in_=dbc)
                hp = acc_all[0:C, 512:1024]
                nc.tensor.matmul(
                    out=hp, lhsT=wwt, rhs=nsb[:, sl],
                    start=True, stop=True, skip_group_check=True,
                )
                nc.vector.tensor_tensor(
                    out=h[:, qoff + 512 * c:qoff + 512 * c + 512],
                    in0=hp, in1=rec, op=ALU.mult,
                )

            # per-half BN statistics (Square stays in the exp table set)
            hv = h[:, qoff:qoff + 1024]
            nc.vector.tensor_reduce(
                out=sh[:, hh:hh + 1], in_=hv,
                axis=mybir.AxisListType.X, op=ALU.add,
            )
            nc.vector.scalar_tensor_tensor(
                out=junk, in0=hv, scalar=1.0, in1=hv,
                op0=ALU.mult, op1=ALU.mult,
                accum_out=sq[:, hh:hh + 1],
            )

            if hh == 0:
                # AR#1: half-A stats + the halo edge row (local row 31)
                pay1 = work.tile([C, 68], f32, tag="pay1")
                nc.gpsimd.tensor_copy(out=pay1[:, 0:1], in_=sh[:, 0:1])
                nc.gpsimd.tensor_copy(out=pay1[:, 1:2], in_=sq[:, 0:1])
                nc.gpsimd.tensor_copy(out=pay1[:, 2:66], in_=h[:, NQ - 64:NQ])
                nc.vector.memset(pay1[:, 66:68], 0.0)
                nc.gpsimd.dma_start(out=ag1_in, in_=pay1)
                nc.gpsimd.collective_compute(
                    "AllGather", ALU.bypass,
                    replica_groups=[CORES],
                    ins=[ag1_in[:].opt()], outs=[ag1_out[:].opt()],
                )
                g1 = work.tile([C, 8, 68], f32, tag="g1")
                nc.sync.dma_start(
                    out=g1, in_=ag1_out.rearrange("(r c) s -> c r s", c=C)
                )
                red1 = work.tile([C, 2], f32, tag="red1")
                nc.vector.tensor_reduce(
                    out=red1, in_=g1[:, :, 0:2].rearrange("c r s -> c s r"),
                    axis=mybir.AxisListType.X, op=ALU.add,
                )
                prod = work.tile([C, 8, 64], f32, tag="prod")
                nc.vector.tensor_tensor(
                    out=prod, in0=g1[:, :, 2:66], in1=mask, op=ALU.mult
                )
                halo = work.tile([C, 64], f32, tag="halo")
                nc.vector.tensor_reduce(
                    out=halo, in_=prod.rearrange("c r w -> c w r"),
                    axis=mybir.AxisListType.X, op=ALU.add,
                )

        # AR#2: half-B stats only
        pay2 = work.tile([C, 2], f32, tag="pay2")
        nc.gpsimd.tensor_copy(out=pay2[:, 0:1], in_=sh[:, 1:2])
        nc.gpsimd.tensor_copy(out=pay2[:, 1:2], in_=sq[:, 1:2])
        nc.gpsimd.dma_start(out=ag2_in, in_=pay2)
        nc.gpsimd.collective_compute(
            "AllGather", ALU.bypass,
            replica_groups=[CORES],
            ins=[ag2_in[:].opt()], outs=[ag2_out[:].opt()],
        )
        g2 = work.tile([C, 8, 2], f32, tag="g2")
        nc.sync.dma_start(out=g2, in_=ag2_out.rearrange("(r c) s -> c r s", c=C))
        red2 = work.tile([C, 2], f32, tag="red2")
        nc.vector.tensor_reduce(
            out=red2, in_=g2.rearrange("c r s -> c s r"),
            axis=mybir.AxisListType.X, op=ALU.add,
        )

        # --- h2: pack h (+halo) into [128 = 64ch x 2 rowgroups, 18, 66] ---
        h2 = work.tile([128, 18, 66], fp16, tag="h2")
        nc.sync.dma_start(
            out=h2[0:C, 1:18, 1:65],
            in_=h[:, 0:17 * 64].rearrange("c (r w) -> c r w", w=64),
        )
        nc.sync.dma_start(out=h2[0:C, 0:1, 1:65],
                          in_=h[:, 0:64].rearrange("c (r w) -> c r w", w=64))
        nc.sync.dma_start(
            out=h2[C:128, 0:17, 1:65],
            in_=h[:, 15 * 64:NQ].rearrange("c (r w) -> c r w", w=64),
        )
        halo16 = work.tile([C, 64], fp16, tag="halo16")
        nc.vector.tensor_copy(out=halo16, in_=halo)
        nc.sync.dma_start(
            out=h2[C:128, 17:18, 1:65],
            in_=halo16.rearrange("c (r w) -> c r w", w=64),
        )
        # column clamp pads
        nc.vector.tensor_copy(out=h2[:, :, 0:1], in_=h2[:, :, 1:2])
        nc.vector.tensor_copy(out=h2[:, :, 65:66], in_=h2[:, :, 64:65])

        # --- 2x bilinear upsample (x16 scale folded into BN affine) ---
        rp = work.tile([128, 32, 66], fp16, tag="rp")
        nc.vector.scalar_tensor_tensor(
            out=rp[:, 0::2, :], in0=h2[:, 1:17, :], scalar=3.0,
            in1=h2[:, 0:16, :], op0=ALU.mult, op1=ALU.add,
        )
        nc.vector.scalar_tensor_tensor(
            out=rp[:, 1::2, :], in0=h2[:, 1:17, :], scalar=3.0,
            in1=h2[:, 2:18, :], op0=ALU.mult, op1=ALU.add,
        )
        u = work.tile([128, 32, HM], fp16, tag="u")
        nc.vector.scalar_tensor_tensor(
            out=u[:, :, 0::2], in0=rp[:, :, 1:65], scalar=3.0,
            in1=rp[:, :, 0:64], op0=ALU.mult, op1=ALU.add,
        )
        nc.vector.scalar_tensor_tensor(
            out=u[:, :, 1::2], in0=rp[:, :, 1:65], scalar=3.0,
            in1=rp[:, :, 2:66], op0=ALU.mult, op1=ALU.add,
        )

        # --- global BN stats -> affine (post AR#2) ---
        # warm the sqrt table while AR#2 is in flight
        nc.scalar.activation(out=warm, in_=warm, func=AF.Sqrt)
        red = work.tile([C, 2], f32, tag="red")
        nc.vector.tensor_tensor(out=red, in0=red1, in1=red2, op=ALU.add)
        mom = work.tile([C, 2], f32, tag="mom")
        nc.vector.tensor_scalar(
            out=mom, in0=red, scalar1=1.0 / NSTAT, scalar2=None, op0=ALU.mult
        )
        msq = work.tile([C, 1], f32, tag="msq")
        nc.vector.tensor_tensor(
            out=msq, in0=mom[:, 0:1], in1=mom[:, 0:1], op=ALU.mult
        )
        var = work.tile([C, 1], f32, tag="var")
        nc.vector.tensor_tensor(
            out=var, in0=mom[:, 1:2], in1=msq, op=ALU.subtract
        )
        varep = work.tile([C, 1], f32, tag="varep")
        nc.vector.tensor_scalar(
            out=varep, in0=var, scalar1=BN_EPS, scalar2=None, op0=ALU.add
        )
        sd = work.tile([C, 1], f32, tag="sd")
        nc.scalar.activation(out=sd, in_=varep, func=AF.Sqrt)
        isd = work.tile([C, 1], f32, tag="isd")
        nc.vector.reciprocal(out=isd, in_=sd)
        sfull = work.tile([C, 1], f32, tag="sfull")
        nc.vector.tensor_tensor(out=sfull, in0=isd, in1=gam, op=ALU.mult)
        s16 = work.tile([C, 1], f32, tag="s16")
        nc.vector.tensor_tensor(out=s16, in0=isd, in1=g16, op=ALU.mult)
        mt = work.tile([C, 1], f32, tag="mt")
        nc.vector.tensor_tensor(out=mt, in0=mom[:, 0:1], in1=sfull, op=ALU.mult)
        tsh = work.tile([C, 1], f32, tag="tsh")
        nc.vector.tensor_tensor(out=tsh, in0=bet, in1=mt, op=ALU.subtract)
        s2 = work.tile([128, 1], f32, tag="s2")
        nc.sync.dma_start(out=s2[0:C, :], in_=s16)
        nc.sync.dma_start(out=s2[C:128, :], in_=s16)
        t2 = work.tile([128, 1], f32, tag="t2")
        nc.sync.dma_start(out=t2[0:C, :], in_=tsh)
        nc.sync.dma_start(out=t2[C:128, :], in_=tsh)

        # --- out = s*u + t + main ---
        bnu = work.tile([128, 32 * HM], fp16, tag="bnu")
        nc.vector.tensor_scalar(
            out=bnu, in0=u.rearrange("p r w -> p (r w)"),
            scalar1=s2, scalar2=t2, op0=ALU.mult, op1=ALU.add,
        )
        outsb = work.tile([128, 32 * HM], fp16, tag="outsb")
        nc.vector.tensor_tensor(out=outsb, in0=bnu, in1=mainr, op=ALU.add)
        nc.sync.dma_start(out=d_o, in_=outsb)

    nc.compile()
    return nc


def _host_prep(main_feature, cross_feature, g_w, g_b, theta_w, theta_b,
               phi_w, phi_b, w_w, w_b, bn_gamma, bn_beta):
    """Build all per-core device inputs on the host (f64 where cheap)."""
    per_batch = []
    for b in range(B):
        cr = cross_feature[b].reshape(C, NC).astype(np.float64)
        y = theta_w.astype(np.float64) @ cr                 # [32, 4096]
        x = g_w.astype(np.float64) @ cr                     # [32, 4096]
        r = (g_b.astype(np.float64) @ y)                    # [4096]
        m = main_feature[b]
        pm = 0.25 * (m[:, 0::2, 0::2] + m[:, 0::2, 1::2]
                     + m[:, 1::2, 0::2] + m[:, 1::2, 1::2])
        z = phi_w.astype(np.float64) @ pm.reshape(C, NC).astype(np.float64)
        z = z + phi_b.astype(np.float64)[:, None]           # [32, 4096]

        y4 = np.tile(y.astype(np.float32), (4, 1)).astype(NPBF16)
        # fold the softmax-invariant bias r into z: es rows get scaled by
        # e^r via the z matmul (incl. the denominator "ones" column)
        er = np.exp(r)                                      # [4096]
        zt = np.empty((128, NT, 33), dtype=np.float64)
        zt[:, :, 0] = 1.0
        zt[:, :, 1:] = z.T.reshape(NT, 128, CI).transpose(1, 0, 2)
        zt *= er.reshape(NT, 128).T[:, :, None]
        per_batch.append((x, y4, zt.astype(NPBF16)))

    wwt = np.concatenate([w_b[None, :], w_w.T], axis=0).astype(np.float32)
    ones64 = np.ones((1, C), dtype=np.float32)
    g16 = (bn_gamma / 16.0).astype(np.float32)[:, None]
    gam = bn_gamma.astype(np.float32)[:, None]
    bet = bn_beta.astype(np.float32)[:, None]

    in_maps = []
    for core in CORES:
        b, qh = core // 2, core % 2
        x, y4, zt = per_batch[b]
        # local row l -> global row g: qh=0: g=l ; qh=1: g=63-l
        if qh == 0:
            rows = np.arange(32)
        else:
            rows = 63 - np.arange(32)
        xq = x.reshape(CI, HC, HC)[:, rows, :].reshape(CI, NQ).astype(np.float32)
        x4 = np.tile(xq, (4, 1)).astype(NPBF16)
        # residual main rows in packed [64*rg + c, i, w] layout
        orow = np.arange(64) if qh == 0 else 127 - np.arange(64)
        mr = main_feature[b][:, orow, :].astype(np.float16)      # [64, 64, 128]
        mainr = mr.reshape(C, 2, 32, HM).transpose(1, 0, 2, 3).reshape(
            128, 32 * HM)
        mask = np.zeros((C, 8 * 64), dtype=np.float32)
        partner = core ^ 1
        mask[:, partner * 64:(partner + 1) * 64] = 1.0
        in_maps.append({
            "y4": y4, "x4": x4,
            "zaug": zt.reshape(128, NT * 33),
            "wwt": wwt, "ones64": ones64, "mask": mask,
            "mainr": mainr, "g16": g16, "gam": gam, "bet": bet,
        })
    return in_maps


def kernel(main_feature, cross_feature, g_w, g_b, theta_w, theta_b,
           phi_w, phi_b, w_w, w_b, bn_gamma, bn_beta):
    main_feature = np.ascontiguousarray(main_feature, dtype=np.float32)
    cross_feature = np.ascontiguousarray(cross_feature, dtype=np.float32)

    if "k" not in _cache:
        _cache["k"] = _build()

    in_maps = _host_prep(main_feature, cross_feature, g_w, g_b, theta_w,
                         theta_b, phi_w, phi_b, w_w, w_b, bn_gamma, bn_beta)
    res = _run(_cache["k"], in_maps, "k")

    out = np.empty((B, C, HM, HM), dtype=np.float32)
    for core in CORES:
        b, qh = core // 2, core % 2
        v = res[core]["outp"].astype(np.float32).reshape(2, C, 32, HM)
        v = v.transpose(1, 0, 2, 3).reshape(C, 64, HM)
        if qh == 0:
            out[b][:, 0:64, :] = v
        else:
            out[b][:, 64:128, :] = v[:, ::-1, :]
    return out


# revision 20
# speedup vs baseline: 1.0849x; 1.0849x over previous
"""Trainium2 Bass kernel for nn_CrossLocal (cross-attention + convs + BN +
bilinear resizes), distributed over 8 NeuronCores in a SINGLE fused launch.

Sharding: core = 2*b + qh owns half the query rows of batch b's 64x64 cross
image (32 rows = 2048 queries).  For qh=1 the row axis is mirrored on the
host so every core runs the identical program ("outer edge" at local row 0,
cross-core halo after local row 31).

Host precomputes (tiny, <<1ms of matmul): y = theta_w@cr, x = g_w@cr,
r[m] = g_b.y_m (softmax-invariant bias), z = phi(avgpool2x2(main)) with an
extra ones column that accumulates the softmax denominator for free.

Device pipeline per core:
  S_T[m-tile, q] = y_tile^T @ x  (K=32 bf16 matmuls, 4x PE row-tiling)
  es = exp(S + r): even m-tiles on ScalarE (table exp), odd m-tiles on DVE
       via the Schraudolph bit-trick (round(a*S + b) as int16 = bf16 bits
       of 2^(S+r)/log-scale; softmax averaging washes out the ~3.5% error)
  num/den = z_aug^T @ es accumulated in PSUM (K=128 bf16)
  h = (W@num + b*den)/den via ones-broadcast + reciprocal
  BN stats: two pipelined AllGathers (half A carries the halo edge row so
  the collective overlaps the second half's compute), affine folded into
  the 2x bilinear upsample output, + residual, fp16 I/O throughout.
"""

import os
from contextlib import ExitStack

import numpy as np
import ml_dtypes

import concourse.bass as bass
import concourse.tile as tile
from concourse import bacc, mybir
from concourse.bass_utils import run_bass_kernel_spmd

f32 = mybir.dt.float32
f32r = mybir.dt.float32r
fp16 = mybir.dt.float16
bf16 = mybir.dt.bfloat16
i16 = mybir.dt.int16
AF = mybir.ActivationFunctionType
ALU = mybir.AluOpType

NPBF16 = ml_dtypes.bfloat16

B, C, CI = 4, 64, 32
HM, HC = 128, 64
NC = HC * HC          # 4096 keys
NQ = NC // 2          # 2048 queries per core
NT = NC // 128        # 32 m-tiles
BN_EPS = 1e-5
NSTAT = float(B * NC)
CORES = list(range(8))

# Schraudolph 2^x constants for bf16 bit construction (7 mantissa bits):
# bits = round(A*(S + r) + CC);  A = 128*log2(e), CC = 16256 - 128*delta*
A_EXP = 128 * 1.4426950408889634
C_EXP = 16251.4432

_cache = {}
last_profile = {}


def _run(nc, in_maps, name):
    trace = os.environ.get("BASS_PROFILE", "") == "1"
    tmpdir = None
    if trace:
        tmpdir = os.path.join("/tmp/bass_traces", name)
        os.makedirs(tmpdir, exist_ok=True)
    br = run_bass_kernel_spmd(
        nc, in_maps, core_ids=CORES, trace=trace, tmpdir=tmpdir
    )
    if trace:
        last_profile[name] = br
    return br.results


def _build():
    nc = bacc.Bacc("TRN2", target_bir_lowering=False, debug=False,
                   num_devices=len(CORES))
    d_y4 = nc.dram_tensor("y4", [128, NC], bf16, kind="ExternalInput").ap()
    d_x4 = nc.dram_tensor("x4", [128, NQ], bf16, kind="ExternalInput").ap()
    d_z = nc.dram_tensor("zaug", [128, NT * 33], bf16, kind="ExternalInput").ap()
    d_wwt = nc.dram_tensor("wwt", [33, C], f32, kind="ExternalInput").ap()
    d_ones = nc.dram_tensor("ones64", [1, C], f32, kind="ExternalInput").ap()
    d_mask = nc.dram_tensor("mask", [C, 8 * 64], f32, kind="ExternalInput").ap()
    d_mn = nc.dram_tensor("mainr", [128, 32 * HM], fp16, kind="ExternalInput").ap()
    d_g16 = nc.dram_tensor("g16", [C, 1], f32, kind="ExternalInput").ap()
    d_gam = nc.dram_tensor("gam", [C, 1], f32, kind="ExternalInput").ap()
    d_bet = nc.dram_tensor("bet", [C, 1], f32, kind="ExternalInput").ap()
    d_o = nc.dram_tensor("outp", [128, 32 * HM], fp16, kind="ExternalOutput").ap()

    with ExitStack() as ctx:
        tc = ctx.enter_context(tile.TileContext(nc))
        const = ctx.enter_context(tc.tile_pool(name="const", bufs=1))
        work = ctx.enter_context(tc.tile_pool(name="work", bufs=1))
        half = ctx.enter_context(tc.tile_pool(name="half", bufs=2))
        es_s = ctx.enter_context(tc.tile_pool(name="es_s", bufs=6))
        ps_s = ctx.enter_context(tc.tile_pool(name="ps_s", bufs=3, space="PSUM"))
        ps_acc = ctx.enter_context(tc.tile_pool(name="ps_acc", bufs=1, space="PSUM"))
        dram = ctx.enter_context(tc.tile_pool(name="dram", bufs=1, space="DRAM"))

        # --- warm the exp table ASAP ---
        warm = const.tile([1, 1], f32, tag="warm")
        nc.vector.memset(warm, 0.0)
        nc.scalar.activation(out=warm, in_=warm, func=AF.Exp)

        # --- force the PE HAM throttle to K=8/8 with a dense dummy burst.
        # The steady-state S/exp/attnz pipeline is fast enough to KEEP the
        # PE warm but too gappy to WARM it from cold (bistable hysteresis);
        # ~5us of back-to-back accumulating matmuls tips it over.
        wrow = const.tile([1, 512], bf16, tag="wrow")
        nc.vector.memset(wrow, 1.0)
        wone = const.tile([1, C], bf16, tag="wone")
        nc.vector.memset(wone, 1.0)

        # --- constant loads ---
        y4 = const.tile([128, NC], bf16, tag="y4")
        nc.sync.dma_start(out=y4, in_=d_y4)
        x4 = const.tile([128, NQ], bf16, tag="x4")
        nc.sync.dma_start(out=x4, in_=d_x4)
        z_sb = const.tile([128, NT, 33], bf16, tag="z_sb")
        nc.sync.dma_start(out=z_sb, in_=d_z.rearrange("p (t s) -> p t s", s=33))
        wwt = const.tile([33, C], f32r, tag="wwt")
        nc.gpsimd.dma_start(out=wwt, in_=d_wwt)
        ones64 = const.tile([1, C], f32r, tag="ones64")
        nc.gpsimd.dma_start(out=ones64, in_=d_ones)
        mask = const.tile([C, 8, 64], f32, tag="mask")
        nc.sync.dma_start(out=mask, in_=d_mask.rearrange("c (r w) -> c r w", w=64))
        mainr = const.tile([128, 32 * HM], fp16, tag="mainr")
        nc.gpsimd.dma_start(out=mainr, in_=d_mn)
        g16 = const.tile([C, 1], f32, tag="g16")
        nc.sync.dma_start(out=g16, in_=d_g16)
        gam = const.tile([C, 1], f32, tag="gam")
        nc.sync.dma_start(out=gam, in_=d_gam)
        bet = const.tile([C, 1], f32, tag="bet")
        nc.sync.dma_start(out=bet, in_=d_bet)

        # --- persistent work tiles ---
        h = work.tile([C, NQ], fp16, tag="h")           # local q order
        junk = work.tile([C, 1024], fp16, tag="junk")   # gpsimd stt scratch
        sh = work.tile([C, 2], f32, tag="sh")           # per-half sum(h)
        sq = work.tile([C, 2], f32, tag="sq")           # per-half sum(h^2)
        acc_all = ps_acc.tile([128, 1024], f32, tag="acc")

        # DRAM bounce buffers for the two AllGathers
        ag1_in = dram.tile([C, 68], f32)
        ag1_out = dram.tile([8 * C, 68], f32)
        ag2_in = dram.tile([C, 2], f32)
        ag2_out = dram.tile([8 * C, 2], f32)

        # attention: halves are processed edge-rows-first (half A = local
        # rows 16..31 = cols 1024:2048) so the halo row ships with AR#1.
        def emit_az(hh, t, es_bf):
            accp = acc_all[64 * hh:64 * hh + 33, :]
            for c in range(2):
                nc.tensor.matmul(
                    out=accp[:, 512 * c:512 * c + 512],
                    lhsT=z_sb[:, t, :],
                    rhs=es_bf[:, 512 * c:512 * c + 512],
                    start=(t == 0), stop=(t == NT - 1),
                    skip_group_check=True,
                    tile_position=(0, 64 * hh),
                )

        # HAM warm-up: one long accumulation group streams gap-free
        for wi in range(14):
            nc.tensor.matmul(
                out=acc_all[0:C, 0:512], lhsT=wone, rhs=wrow,
                start=(wi == 0), stop=(wi == 13), skip_group_check=True,
            )

        batches = [list(range(b, min(b + 3, NT))) for b in range(0, NT, 3)]
        for hh, qoff in ((0, 1024), (1, 0)):
            pend = []
            for bts in batches:
                sps = {}
                for t in bts:
                    j = t % 4
                    s_ps = ps_s.tile([128, 1024], f32, tag="s")
                    sps[t] = s_ps
                    for k in range(2):
                        nc.tensor.matmul(
                            out=s_ps[:, 512 * k:512 * k + 512],
                            lhsT=y4[32 * j:32 * j + 32, 128 * t:128 * t + 128],
                            rhs=x4[32 * j:32 * j + 32,
                                   qoff + 512 * k:qoff + 512 * k + 512],
                            start=True, stop=True,
                            skip_group_check=True,
                            tile_position=(32 * j, 0),
                        )
                for t in bts:
                    es = es_s.tile([128, 1024], bf16, tag="es")
                    nc.scalar.activation(out=es, in_=sps[t], func=AF.Exp)
                    pend.append((t, es))
                # drain attn@z one batch behind so the PE never waits on
                # an exp that was just issued
                while len(pend) > len(bts):
                    td, esd = pend.pop(0)
                    emit_az(hh, td, esd)
            for (td, esd) in pend:
                emit_az(hh, td, esd)

            # --- epilogue for this half ---
            accp = acc_all[64 * hh:64 * hh + 33, :]
            nsb = half.tile([33, 1024], f32r, tag="nsb")
            nc.vector.tensor_copy(out=nsb, in_=accp)
            for c in range(2):
                sl = slice(512 * c, 512 * c + 512)
                dbc = acc_all[0:C, 0:512]
                nc.tensor.matmul(
                    out=dbc, lhsT=ones64, rhs=nsb[0:1, sl],
                    start=True, stop=True, skip_group_check=True,
                )
                rec = half.tile([C, 512], f32, tag="rec")
                nc.vector.reciprocal_approx_fast(out=rec, in_=dbc)
                hp = acc_all[0:C, 512:1024]
                nc.tensor.matmul(
                    out=hp, lhsT=wwt, rhs=nsb[:, sl],
                    start=True, stop=True, skip_group_check=True,
                )
                nc.vector.tensor_tensor(
                    out=h[:, qoff + 512 * c:qoff + 512 * c + 512],
                    in0=hp, in1=rec, op=ALU.mult,
                )

            # per-half BN statistics (Square stays in the exp table set)
            hv = h[:, qoff:qoff + 1024]
            nc.vector.tensor_reduce(
                out=sh[:, hh:hh + 1], in_=hv,
                axis=mybir.AxisListType.X, op=ALU.add,
            )
            nc.vector.scalar_tensor_tensor(
                out=junk, in0=hv, scalar=1.0, in1=hv,
                op0=ALU.mult, op1=ALU.mult,
                accum_out=sq[:, hh:hh + 1],
            )

            if hh == 0:
                # AR#1: half-A stats + the halo edge row (local row 31)
                pay1 = work.tile([C, 68], f32, tag="pay1")
                nc.gpsimd.tensor_copy(out=pay1[:, 0:1], in_=sh[:, 0:1])
                nc.gpsimd.tensor_copy(out=pay1[:, 1:2], in_=sq[:, 0:1])
                nc.gpsimd.tensor_copy(out=pay1[:, 2:66], in_=h[:, NQ - 64:NQ])
                nc.vector.memset(pay1[:, 66:68], 0.0)
                nc.gpsimd.dma_start(out=ag1_in, in_=pay1)
                nc.gpsimd.collective_compute(
                    "AllGather", ALU.bypass,
                    replica_groups=[CORES],
                    ins=[ag1_in[:].opt()], outs=[ag1_out[:].opt()],
                )
                g1 = work.tile([C, 8, 68], f32, tag="g1")
                nc.sync.dma_start(
                    out=g1, in_=ag1_out.rearrange("(r c) s -> c r s", c=C)
                )
                red1 = work.tile([C, 2], f32, tag="red1")
                nc.vector.tensor_reduce(
                    out=red1, in_=g1[:, :, 0:2].rearrange("c r s -> c s r"),
                    axis=mybir.AxisListType.X, op=ALU.add,
                )
                prod = work.tile([C, 8, 64], f32, tag="prod")
                nc.vector.tensor_tensor(
                    out=prod, in0=g1[:, :, 2:66], in1=mask, op=ALU.mult
                )
                halo = work.tile([C, 64], f32, tag="halo")
                nc.vector.tensor_reduce(
                    out=halo, in_=prod.rearrange("c r w -> c w r"),
                    axis=mybir.AxisListType.X, op=ALU.add,
                )

        # AR#2: half-B stats only
        pay2 = work.tile([C, 2], f32, tag="pay2")
        nc.gpsimd.tensor_copy(out=pay2[:, 0:1], in_=sh[:, 1:2])
        nc.gpsimd.tensor_copy(out=pay2[:, 1:2], in_=sq[:, 1:2])
        nc.gpsimd.dma_start(out=ag2_in, in_=pay2)
        nc.gpsimd.collective_compute(
            "AllGather", ALU.bypass,
            replica_groups=[CORES],
            ins=[ag2_in[:].opt()], outs=[ag2_out[:].opt()],
        )
        g2 = work.tile([C, 8, 2], f32, tag="g2")
        nc.sync.dma_start(out=g2, in_=ag2_out.rearrange("(r c) s -> c r s", c=C))
        red2 = work.tile([C, 2], f32, tag="red2")
        nc.vector.tensor_reduce(
            out=red2, in_=g2.rearrange("c r s -> c s r"),
            axis=mybir.AxisListType.X, op=ALU.add,
        )

        # --- h2: pack h (+halo) into [128 = 64ch x 2 rowgroups, 18, 66] ---
        h2 = work.tile([128, 18, 66], fp16, tag="h2")
        nc.sync.dma_start(
            out=h2[0:C, 1:18, 1:65],
            in_=h[:, 0:17 * 64].rearrange("c (r w) -> c r w", w=64),
        )
        nc.sync.dma_start(out=h2[0:C, 0:1, 1:65],
                          in_=h[:, 0:64].rearrange("c (r w) -> c r w", w=64))
        nc.sync.dma_start(
            out=h2[C:128, 0:17, 1:65],
            in_=h[:, 15 * 64:NQ].rearrange("c (r w) -> c r w", w=64),
        )
        halo16 = work.tile([C, 64], fp16, tag="halo16")
        nc.vector.tensor_copy(out=halo16, in_=halo)
        nc.sync.dma_start(
            out=h2[C:128, 17:18, 1:65],
            in_=halo16.rearrange("c (r w) -> c r w", w=64),
        )
        # column clamp pads
        nc.vector.tensor_copy(out=h2[:, :, 0:1], in_=h2[:, :, 1:2])
        nc.vector.tensor_copy(out=h2[:, :, 65:66], in_=h2[:, :, 64:65])

        # --- 2x bilinear upsample (x16 scale folded into BN affine) ---
        rp = work.tile([128, 32, 66], fp16, tag="rp")
        nc.vector.scalar_tensor_tensor(
            out=rp[:, 0::2, :], in0=h2[:, 1:17, :], scalar=3.0,
            in1=h2[:, 0:16, :], op0=ALU.mult, op1=ALU.add,
        )
        nc.vector.scalar_tensor_tensor(
            out=rp[:, 1::2, :], in0=h2[:, 1:17, :], scalar=3.0,
            in1=h2[:, 2:18, :], op0=ALU.mult, op1=ALU.add,
        )
        u = work.tile([128, 32, HM], fp16, tag="u")
        nc.vector.scalar_tensor_tensor(
            out=u[:, :, 0::2], in0=rp[:, :, 1:65], scalar=3.0,
            in1=rp[:, :, 0:64], op0=ALU.mult, op1=ALU.add,
        )
        nc.vector.scalar_tensor_tensor(
            out=u[:, :, 1::2], in0=rp[:, :, 1:65], scalar=3.0,
            in1=rp[:, :, 2:66], op0=ALU.mult, op1=ALU.add,
        )

        # --- global BN stats -> affine (post AR#2) ---
        # warm the sqrt table while AR#2 is in flight
        nc.scalar.activation(out=warm, in_=warm, func=AF.Sqrt)
        red = work.tile([C, 2], f32, tag="red")
        nc.vector.tensor_tensor(out=red, in0=red1, in1=red2, op=ALU.add)
        mom = work.tile([C, 2], f32, tag="mom")
        nc.vector.tensor_scalar(
            out=mom, in0=red, scalar1=1.0 / NSTAT, scalar2=None, op0=ALU.mult
        )
        msq = work.tile([C, 1], f32, tag="msq")
        nc.vector.tensor_tensor(
            out=msq, in0=mom[:, 0:1], in1=mom[:, 0:1], op=ALU.mult
        )
        var = work.tile([C, 1], f32, tag="var")
        nc.vector.tensor_tensor(
            out=var, in0=mom[:, 1:2], in1=msq, op=ALU.subtract
        )
        varep = work.tile([C, 1], f32, tag="varep")
        nc.vector.tensor_scalar(
            out=varep, in0=var, scalar1=BN_EPS, scalar2=None, op0=ALU.add
        )
        sd = work.tile([C, 1], f32, tag="sd")
        nc.scalar.activation(out=sd, in_=varep, func=AF.Sqrt)
        isd = work.tile([C, 1], f32, tag="isd")
        nc.vector.reciprocal(out=isd, in_=sd)
        sfull = work.tile([C, 1], f32, tag="sfull")
        nc.vector.tensor_tensor(out=sfull, in0=isd, in1=gam, op=ALU.mult)
        s16 = work.tile([C, 1], f32, tag="s16")
        nc.vector.tensor_tensor(out=s16, in0=isd, in1=g16, op=ALU.mult)
        mt = work.tile([C, 1], f32, tag="mt")
        nc.vector.tensor_tensor(out=mt, in0=mom[:, 0:1], in1=sfull, op=ALU.mult)
        tsh = work.tile([C, 1], f32, tag="tsh")
        nc.vector.tensor_tensor(out=tsh, in0=bet, in1=mt, op=ALU.subtract)
        s2 = work.tile([128, 1], f32, tag="s2")
        nc.sync.dma_start(out=s2[0:C, :], in_=s16)
        nc.sync.dma_start(out=s2[C:128, :], in_=s16)
        t2 = work.tile([128, 1], f32, tag="t2")
        nc.sync.dma_start(out=t2[0:C, :], in_=tsh)
        nc.sync.dma_start(out=t2[C:128, :], in_=tsh)

        # --- out = s*u + t + main ---
        bnu = work.tile([128, 32 * HM], fp16, tag="bnu")
        nc.vector.tensor_scalar(
            out=bnu, in0=u.rearrange("p r w -> p (r w)"),
            scalar1=s2, scalar2=t2, op0=ALU.mult, op1=ALU.add,
        )
        outsb = work.tile([128, 32 * HM], fp16, tag="outsb")
        nc.vector.tensor_tensor(out=outsb, in0=bnu, in1=mainr, op=ALU.add)
        nc.sync.dma_start(out=d_o, in_=outsb)

    nc.compile()
    return nc


def _host_prep(main_feature, cross_feature, g_w, g_b, theta_w, theta_b,
               phi_w, phi_b, w_w, w_b, bn_gamma, bn_beta):
    """Build all per-core device inputs on the host (f64 where cheap)."""
    per_batch = []
    for b in range(B):
        cr = cross_feature[b].reshape(C, NC).astype(np.float64)
        y = theta_w.astype(np.float64) @ cr                 # [32, 4096]
        x = g_w.astype(np.float64) @ cr                     # [32, 4096]
        r = (g_b.astype(np.float64) @ y)                    # [4096]
        m = main_feature[b]
        pm = 0.25 * (m[:, 0::2, 0::2] + m[:, 0::2, 1::2]
                     + m[:, 1::2, 0::2] + m[:, 1::2, 1::2])
        z = phi_w.astype(np.float64) @ pm.reshape(C, NC).astype(np.float64)
        z = z + phi_b.astype(np.float64)[:, None]           # [32, 4096]

        y4 = np.tile(y.astype(np.float32), (4, 1)).astype(NPBF16)
        # fold the softmax-invariant bias r into z: es rows get scaled by
        # e^r via the z matmul (incl. the denominator "ones" column)
        er = np.exp(r)                                      # [4096]
        zt = np.empty((128, NT, 33), dtype=np.float64)
        zt[:, :, 0] = 1.0
        zt[:, :, 1:] = z.T.reshape(NT, 128, CI).transpose(1, 0, 2)
        zt *= er.reshape(NT, 128).T[:, :, None]
        per_batch.append((x, y4, zt.astype(NPBF16)))

    wwt = np.concatenate([w_b[None, :], w_w.T], axis=0).astype(np.float32)
    ones64 = np.ones((1, C), dtype=np.float32)
    g16 = (bn_gamma / 16.0).astype(np.float32)[:, None]
    gam = bn_gamma.astype(np.float32)[:, None]
    bet = bn_beta.astype(np.float32)[:, None]

    in_maps = []
    for core in CORES:
        b, qh = core // 2, core % 2
        x, y4, zt = per_batch[b]
        # local row l -> global row g: qh=0: g=l ; qh=1: g=63-l
        if qh == 0:
            rows = np.arange(32)
        else:
            rows = 63 - np.arange(32)
        xq = x.reshape(CI, HC, HC)[:, rows, :].reshape(CI, NQ).astype(np.float32)
        x4 = np.tile(xq, (4, 1)).astype(NPBF16)
        # residual main rows in packed [64*rg + c, i, w] layout
        orow = np.arange(64) if qh == 0 else 127 - np.arange(64)
        mr = main_feature[b][:, orow, :].astype(np.float16)      # [64, 64, 128]
        mainr = mr.reshape(C, 2, 32, HM).transpose(1, 0, 2, 3).reshape(
            128, 32 * HM)
        mask = np.zeros((C, 8 * 64), dtype=np.float32)
        partner = core ^ 1
        mask[:, partner * 64:(partner + 1) * 64] = 1.0
        in_maps.append({
            "y4": y4, "x4": x4,
            "zaug": zt.reshape(128, NT * 33),
            "wwt": wwt, "ones64": ones64, "mask": mask,
            "mainr": mainr, "g16": g16, "gam": gam, "bet": bet,
        })
    return in_maps


def kernel(main_feature, cross_feature, g_w, g_b, theta_w, theta_b,
           phi_w, phi_b, w_w, w_b, bn_gamma, bn_beta):
    main_feature = np.ascontiguousarray(main_feature, dtype=np.float32)
    cross_feature = np.ascontiguousarray(cross_feature, dtype=np.float32)

    if "k" not in _cache:
        _cache["k"] = _build()

    in_maps = _host_prep(main_feature, cross_feature, g_w, g_b, theta_w,
                         theta_b, phi_w, phi_b, w_w, w_b, bn_gamma, bn_beta)
    res = _run(_cache["k"], in_maps, "k")

    out = np.empty((B, C, HM, HM), dtype=np.float32)
    for core in CORES:
        b, qh = core // 2, core % 2
        v = res[core]["outp"].astype(np.float32).reshape(2, C, 32, HM)
        v = v.transpose(1, 0, 2, 3).reshape(C, 64, HM)
        if qh == 0:
            out[b][:, 0:64, :] = v
        else:
            out[b][:, 64:128, :] = v[:, ::-1, :]
    return out


# revision 21
# speedup vs baseline: 1.1249x; 1.0368x over previous
"""Trainium2 Bass kernel for nn_CrossLocal (cross-attention + convs + BN +
bilinear resizes), distributed over 8 NeuronCores in a SINGLE fused launch.

Sharding: core = 2*b + qh owns half the query rows of batch b's 64x64 cross
image (32 rows = 2048 queries).  For qh=1 the row axis is mirrored on the
host so every core runs the identical program ("outer edge" at local row 0,
cross-core halo after local row 31).

Host precomputes (tiny, <<1ms of matmul): y = theta_w@cr, x = g_w@cr,
r[m] = g_b.y_m (softmax-invariant bias), z = phi(avgpool2x2(main)) with an
extra ones column that accumulates the softmax denominator for free.

Device pipeline per core:
  S_T[m-tile, q] = y_tile^T @ x  (K=32 bf16 matmuls, 4x PE row-tiling)
  es = exp(S + r): even m-tiles on ScalarE (table exp), odd m-tiles on DVE
       via the Schraudolph bit-trick (round(a*S + b) as int16 = bf16 bits
       of 2^(S+r)/log-scale; softmax averaging washes out the ~3.5% error)
  num/den = z_aug^T @ es accumulated in PSUM (K=128 bf16)
  h = (W@num + b*den)/den via ones-broadcast + reciprocal
  BN stats: two pipelined AllGathers (half A carries the halo edge row so
  the collective overlaps the second half's compute), affine folded into
  the 2x bilinear upsample output, + residual, fp16 I/O throughout.
"""

import os
from contextlib import ExitStack

import numpy as np
import ml_dtypes

import concourse.bass as bass
import concourse.tile as tile
from concourse import bacc, mybir
from concourse.bass_utils import run_bass_kernel_spmd

f32 = mybir.dt.float32
f32r = mybir.dt.float32r
fp16 = mybir.dt.float16
bf16 = mybir.dt.bfloat16
i16 = mybir.dt.int16
AF = mybir.ActivationFunctionType
ALU = mybir.AluOpType

NPBF16 = ml_dtypes.bfloat16

B, C, CI = 4, 64, 32
HM, HC = 128, 64
NC = HC * HC          # 4096 keys
NQ = NC // 2          # 2048 queries per core
NT = NC // 128        # 32 m-tiles
BN_EPS = 1e-5
NSTAT = float(B * NC)
CORES = list(range(8))

# Schraudolph 2^x constants for bf16 bit construction (7 mantissa bits):
# bits = round(A*(S + r) + CC);  A = 128*log2(e), CC = 16256 - 128*delta*
A_EXP = 128 * 1.4426950408889634
C_EXP = 16251.4432

_cache = {}
last_profile = {}


def _run(nc, in_maps, name):
    trace = os.environ.get("BASS_PROFILE", "") == "1"
    tmpdir = None
    if trace:
        tmpdir = os.path.join("/tmp/bass_traces", name)
        os.makedirs(tmpdir, exist_ok=True)
    br = run_bass_kernel_spmd(
        nc, in_maps, core_ids=CORES, trace=trace, tmpdir=tmpdir
    )
    if trace:
        last_profile[name] = br
    return br.results


def _build():
    nc = bacc.Bacc("TRN2", target_bir_lowering=False, debug=False,
                   num_devices=len(CORES))
    d_y4 = nc.dram_tensor("y4", [128, NC], bf16, kind="ExternalInput").ap()
    d_x4 = nc.dram_tensor("x4", [128, NQ], bf16, kind="ExternalInput").ap()
    d_z = nc.dram_tensor("zaug", [128, NT * 33], bf16, kind="ExternalInput").ap()
    d_wwt = nc.dram_tensor("wwt", [33, C], f32, kind="ExternalInput").ap()
    d_ones = nc.dram_tensor("ones64", [1, C], f32, kind="ExternalInput").ap()
    d_mask = nc.dram_tensor("mask", [C, 8 * 64], f32, kind="ExternalInput").ap()
    d_mn = nc.dram_tensor("mainr", [128, 32 * HM], fp16, kind="ExternalInput").ap()
    d_g16 = nc.dram_tensor("g16", [C, 1], f32, kind="ExternalInput").ap()
    d_gam = nc.dram_tensor("gam", [C, 1], f32, kind="ExternalInput").ap()
    d_bet = nc.dram_tensor("bet", [C, 1], f32, kind="ExternalInput").ap()
    d_o = nc.dram_tensor("outp", [128, 32 * HM], fp16, kind="ExternalOutput").ap()

    with ExitStack() as ctx:
        tc = ctx.enter_context(tile.TileContext(nc))
        const = ctx.enter_context(tc.tile_pool(name="const", bufs=1))
        work = ctx.enter_context(tc.tile_pool(name="work", bufs=1))
        half = ctx.enter_context(tc.tile_pool(name="half", bufs=2))
        es_s = ctx.enter_context(tc.tile_pool(name="es_s", bufs=6))
        ps_s = ctx.enter_context(tc.tile_pool(name="ps_s", bufs=3, space="PSUM"))
        ps_acc = ctx.enter_context(tc.tile_pool(name="ps_acc", bufs=1, space="PSUM"))
        dram = ctx.enter_context(tc.tile_pool(name="dram", bufs=1, space="DRAM"))

        # --- warm the exp table ASAP ---
        warm = const.tile([1, 1], f32, tag="warm")
        nc.vector.memset(warm, 0.0)
        nc.scalar.activation(out=warm, in_=warm, func=AF.Exp)

        # --- force the PE HAM throttle to K=8/8 with a dense dummy burst.
        # The steady-state S/exp/attnz pipeline is fast enough to KEEP the
        # PE warm but too gappy to WARM it from cold (bistable hysteresis);
        # ~5us of back-to-back accumulating matmuls tips it over.
        wsrc = const.tile([128, 512], bf16, tag="wsrc")
        nc.vector.memset(wsrc, 1.0)

        # --- constant loads ---
        y4 = const.tile([128, NC], bf16, tag="y4")
        nc.sync.dma_start(out=y4, in_=d_y4)
        x4 = const.tile([128, NQ], bf16, tag="x4")
        nc.sync.dma_start(out=x4, in_=d_x4)
        z_sb = const.tile([128, NT, 33], bf16, tag="z_sb")
        nc.sync.dma_start(out=z_sb, in_=d_z.rearrange("p (t s) -> p t s", s=33))
        wwt = const.tile([33, C], f32r, tag="wwt")
        nc.gpsimd.dma_start(out=wwt, in_=d_wwt)
        ones64 = const.tile([1, C], f32r, tag="ones64")
        nc.gpsimd.dma_start(out=ones64, in_=d_ones)
        mask = const.tile([C, 8, 64], f32, tag="mask")
        nc.sync.dma_start(out=mask, in_=d_mask.rearrange("c (r w) -> c r w", w=64))
        mainr = const.tile([128, 32 * HM], fp16, tag="mainr")
        nc.gpsimd.dma_start(out=mainr, in_=d_mn)
        g16 = const.tile([C, 1], f32, tag="g16")
        nc.sync.dma_start(out=g16, in_=d_g16)
        gam = const.tile([C, 1], f32, tag="gam")
        nc.sync.dma_start(out=gam, in_=d_gam)
        bet = const.tile([C, 1], f32, tag="bet")
        nc.sync.dma_start(out=bet, in_=d_bet)

        # --- persistent work tiles ---
        h = work.tile([C, NQ], fp16, tag="h")           # local q order
        junk = work.tile([C, 1024], fp16, tag="junk")   # gpsimd stt scratch
        sh = work.tile([C, 2], f32, tag="sh")           # per-half sum(h)
        sq = work.tile([C, 2], f32, tag="sq")           # per-half sum(h^2)
        acc_all = ps_acc.tile([128, 1024], f32, tag="acc")

        # DRAM bounce buffers for the two AllGathers
        ag1_in = dram.tile([C, 68], f32)
        ag1_out = dram.tile([8 * C, 68], f32)
        ag2_in = dram.tile([C, 2], f32)
        ag2_out = dram.tile([8 * C, 2], f32)

        # attention: halves are processed edge-rows-first (half A = local
        # rows 16..31 = cols 1024:2048) so the halo row ships with AR#1.
        def emit_az(hh, t, es_bf):
            accp = acc_all[64 * hh:64 * hh + 33, :]
            for c in range(2):
                nc.tensor.matmul(
                    out=accp[:, 512 * c:512 * c + 512],
                    lhsT=z_sb[:, t, :],
                    rhs=es_bf[:, 512 * c:512 * c + 512],
                    start=(t == 0), stop=(t == NT - 1),
                    skip_group_check=True,
                    tile_position=(0, 64 * hh),
                )

        # HAM warm-up: one long K=128 accumulation group streams gap-free
        # with the full array active (K=1 dummies do NOT trip the HAM)
        for wi in range(14):
            nc.tensor.matmul(
                out=acc_all[0:C, 0:512], lhsT=wsrc[:, 0:C], rhs=wsrc,
                start=(wi == 0), stop=(wi == 13), skip_group_check=True,
            )

        batches = [list(range(b, min(b + 3, NT))) for b in range(0, NT, 3)]
        for hh, qoff in ((0, 1024), (1, 0)):
            pend = []
            for bts in batches:
                sps = {}
                for t in bts:
                    j = t % 4
                    s_ps = ps_s.tile([128, 1024], f32, tag="s")
                    sps[t] = s_ps
                    for k in range(2):
                        nc.tensor.matmul(
                            out=s_ps[:, 512 * k:512 * k + 512],
                            lhsT=y4[32 * j:32 * j + 32, 128 * t:128 * t + 128],
                            rhs=x4[32 * j:32 * j + 32,
                                   qoff + 512 * k:qoff + 512 * k + 512],
                            start=True, stop=True,
                            skip_group_check=True,
                            tile_position=(32 * j, 0),
                        )
                for t in bts:
                    es = es_s.tile([128, 1024], bf16, tag="es")
                    nc.scalar.activation(out=es, in_=sps[t], func=AF.Exp)
                    pend.append((t, es))
                # drain attn@z one batch behind so the PE never waits on
                # an exp that was just issued
                while len(pend) > len(bts):
                    td, esd = pend.pop(0)
                    emit_az(hh, td, esd)
            for (td, esd) in pend:
                emit_az(hh, td, esd)

            # --- epilogue for this half ---
            accp = acc_all[64 * hh:64 * hh + 33, :]
            nsb = half.tile([33, 1024], f32r, tag="nsb")
            nc.vector.tensor_copy(out=nsb, in_=accp)
            for c in range(2):
                sl = slice(512 * c, 512 * c + 512)
                dbc = acc_all[0:C, 0:512]
                nc.tensor.matmul(
                    out=dbc, lhsT=ones64, rhs=nsb[0:1, sl],
                    start=True, stop=True, skip_group_check=True,
                )
                rec = half.tile([C, 512], f32, tag="rec")
                nc.vector.reciprocal_approx_fast(out=rec, in_=dbc)
                hp = acc_all[0:C, 512:1024]
                nc.tensor.matmul(
                    out=hp, lhsT=wwt, rhs=nsb[:, sl],
                    start=True, stop=True, skip_group_check=True,
                )
                nc.vector.tensor_tensor(
                    out=h[:, qoff + 512 * c:qoff + 512 * c + 512],
                    in0=hp, in1=rec, op=ALU.mult,
                )

            # per-half BN statistics (Square stays in the exp table set)
            hv = h[:, qoff:qoff + 1024]
            nc.vector.tensor_reduce(
                out=sh[:, hh:hh + 1], in_=hv,
                axis=mybir.AxisListType.X, op=ALU.add,
            )
            nc.vector.scalar_tensor_tensor(
                out=junk, in0=hv, scalar=1.0, in1=hv,
                op0=ALU.mult, op1=ALU.mult,
                accum_out=sq[:, hh:hh + 1],
            )

            if hh == 0:
                # AR#1: half-A stats + the halo edge row (local row 31)
                pay1 = work.tile([C, 68], f32, tag="pay1")
                nc.gpsimd.tensor_copy(out=pay1[:, 0:1], in_=sh[:, 0:1])
                nc.gpsimd.tensor_copy(out=pay1[:, 1:2], in_=sq[:, 0:1])
                nc.gpsimd.tensor_copy(out=pay1[:, 2:66], in_=h[:, NQ - 64:NQ])
                nc.vector.memset(pay1[:, 66:68], 0.0)
                nc.gpsimd.dma_start(out=ag1_in, in_=pay1)
                nc.gpsimd.collective_compute(
                    "AllGather", ALU.bypass,
                    replica_groups=[CORES],
                    ins=[ag1_in[:].opt()], outs=[ag1_out[:].opt()],
                )
                g1 = work.tile([C, 8, 68], f32, tag="g1")
                nc.sync.dma_start(
                    out=g1, in_=ag1_out.rearrange("(r c) s -> c r s", c=C)
                )
                red1 = work.tile([C, 2], f32, tag="red1")
                nc.vector.tensor_reduce(
                    out=red1, in_=g1[:, :, 0:2].rearrange("c r s -> c s r"),
                    axis=mybir.AxisListType.X, op=ALU.add,
                )
                prod = work.tile([C, 8, 64], f32, tag="prod")
                nc.vector.tensor_tensor(
                    out=prod, in0=g1[:, :, 2:66], in1=mask, op=ALU.mult
                )
                halo = work.tile([C, 64], f32, tag="halo")
                nc.vector.tensor_reduce(
                    out=halo, in_=prod.rearrange("c r w -> c w r"),
                    axis=mybir.AxisListType.X, op=ALU.add,
                )

        # AR#2: half-B stats only
        pay2 = work.tile([C, 2], f32, tag="pay2")
        nc.gpsimd.tensor_copy(out=pay2[:, 0:1], in_=sh[:, 1:2])
        nc.gpsimd.tensor_copy(out=pay2[:, 1:2], in_=sq[:, 1:2])
        nc.gpsimd.dma_start(out=ag2_in, in_=pay2)
        nc.gpsimd.collective_compute(
            "AllGather", ALU.bypass,
            replica_groups=[CORES],
            ins=[ag2_in[:].opt()], outs=[ag2_out[:].opt()],
        )
        g2 = work.tile([C, 8, 2], f32, tag="g2")
        nc.sync.dma_start(out=g2, in_=ag2_out.rearrange("(r c) s -> c r s", c=C))
        red2 = work.tile([C, 2], f32, tag="red2")
        nc.vector.tensor_reduce(
            out=red2, in_=g2.rearrange("c r s -> c s r"),
            axis=mybir.AxisListType.X, op=ALU.add,
        )

        # --- h2: pack h (+halo) into [128 = 64ch x 2 rowgroups, 18, 66] ---
        h2 = work.tile([128, 18, 66], fp16, tag="h2")
        nc.sync.dma_start(
            out=h2[0:C, 1:18, 1:65],
            in_=h[:, 0:17 * 64].rearrange("c (r w) -> c r w", w=64),
        )
        nc.sync.dma_start(out=h2[0:C, 0:1, 1:65],
                          in_=h[:, 0:64].rearrange("c (r w) -> c r w", w=64))
        nc.sync.dma_start(
            out=h2[C:128, 0:17, 1:65],
            in_=h[:, 15 * 64:NQ].rearrange("c (r w) -> c r w", w=64),
        )
        halo16 = work.tile([C, 64], fp16, tag="halo16")
        nc.vector.tensor_copy(out=halo16, in_=halo)
        nc.sync.dma_start(
            out=h2[C:128, 17:18, 1:65],
            in_=halo16.rearrange("c (r w) -> c r w", w=64),
        )
        # column clamp pads
        nc.vector.tensor_copy(out=h2[:, :, 0:1], in_=h2[:, :, 1:2])
        nc.vector.tensor_copy(out=h2[:, :, 65:66], in_=h2[:, :, 64:65])

        # --- 2x bilinear upsample (x16 scale folded into BN affine) ---
        rp = work.tile([128, 32, 66], fp16, tag="rp")
        nc.vector.scalar_tensor_tensor(
            out=rp[:, 0::2, :], in0=h2[:, 1:17, :], scalar=3.0,
            in1=h2[:, 0:16, :], op0=ALU.mult, op1=ALU.add,
        )
        nc.vector.scalar_tensor_tensor(
            out=rp[:, 1::2, :], in0=h2[:, 1:17, :], scalar=3.0,
            in1=h2[:, 2:18, :], op0=ALU.mult, op1=ALU.add,
        )
        u = work.tile([128, 32, HM], fp16, tag="u")
        nc.vector.scalar_tensor_tensor(
            out=u[:, :, 0::2], in0=rp[:, :, 1:65], scalar=3.0,
            in1=rp[:, :, 0:64], op0=ALU.mult, op1=ALU.add,
        )
        nc.vector.scalar_tensor_tensor(
            out=u[:, :, 1::2], in0=rp[:, :, 1:65], scalar=3.0,
            in1=rp[:, :, 2:66], op0=ALU.mult, op1=ALU.add,
        )

        # --- global BN stats -> affine (post AR#2) ---
        # warm the sqrt table while AR#2 is in flight
        nc.scalar.activation(out=warm, in_=warm, func=AF.Sqrt)
        red = work.tile([C, 2], f32, tag="red")
        nc.vector.tensor_tensor(out=red, in0=red1, in1=red2, op=ALU.add)
        mom = work.tile([C, 2], f32, tag="mom")
        nc.vector.tensor_scalar(
            out=mom, in0=red, scalar1=1.0 / NSTAT, scalar2=None, op0=ALU.mult
        )
        msq = work.tile([C, 1], f32, tag="msq")
        nc.vector.tensor_tensor(
            out=msq, in0=mom[:, 0:1], in1=mom[:, 0:1], op=ALU.mult
        )
        var = work.tile([C, 1], f32, tag="var")
        nc.vector.tensor_tensor(
            out=var, in0=mom[:, 1:2], in1=msq, op=ALU.subtract
        )
        varep = work.tile([C, 1], f32, tag="varep")
        nc.vector.tensor_scalar(
            out=varep, in0=var, scalar1=BN_EPS, scalar2=None, op0=ALU.add
        )
        sd = work.tile([C, 1], f32, tag="sd")
        nc.scalar.activation(out=sd, in_=varep, func=AF.Sqrt)
        isd = work.tile([C, 1], f32, tag="isd")
        nc.vector.reciprocal(out=isd, in_=sd)
        sfull = work.tile([C, 1], f32, tag="sfull")
        nc.vector.tensor_tensor(out=sfull, in0=isd, in1=gam, op=ALU.mult)
        s16 = work.tile([C, 1], f32, tag="s16")
        nc.vector.tensor_tensor(out=s16, in0=isd, in1=g16, op=ALU.mult)
        mt = work.tile([C, 1], f32, tag="mt")
        nc.vector.tensor_tensor(out=mt, in0=mom[:, 0:1], in1=sfull, op=ALU.mult)
        tsh = work.tile([C, 1], f32, tag="tsh")
        nc.vector.tensor_tensor(out=tsh, in0=bet, in1=mt, op=ALU.subtract)
        s2 = work.tile([128, 1], f32, tag="s2")
        nc.sync.dma_start(out=s2[0:C, :], in_=s16)
        nc.sync.dma_start(out=s2[C:128, :], in_=s16)
        t2 = work.tile([128, 1], f32, tag="t2")
        nc.sync.dma_start(out=t2[0:C, :], in_=tsh)
        nc.sync.dma_start(out=t2[C:128, :], in_=tsh)

        # --- out = s*u + t + main ---
        bnu = work.tile([128, 32 * HM], fp16, tag="bnu")
        nc.vector.tensor_scalar(
            out=bnu, in0=u.rearrange("p r w -> p (r w)"),
            scalar1=s2, scalar2=t2, op0=ALU.mult, op1=ALU.add,
        )
        outsb = work.tile([128, 32 * HM], fp16, tag="outsb")
        nc.vector.tensor_tensor(out=outsb, in0=bnu, in1=mainr, op=ALU.add)
        nc.sync.dma_start(out=d_o, in_=outsb)

    nc.compile()
    return nc


def _host_prep(main_feature, cross_feature, g_w, g_b, theta_w, theta_b,
               phi_w, phi_b, w_w, w_b, bn_gamma, bn_beta):
    """Build all per-core device inputs on the host (f64 where cheap)."""
    per_batch = []
    for b in range(B):
        cr = cross_feature[b].reshape(C, NC).astype(np.float64)
        y = theta_w.astype(np.float64) @ cr                 # [32, 4096]
        x = g_w.astype(np.float64) @ cr                     # [32, 4096]
        r = (g_b.astype(np.float64) @ y)                    # [4096]
        m = main_feature[b]
        pm = 0.25 * (m[:, 0::2, 0::2] + m[:, 0::2, 1::2]
                     + m[:, 1::2, 0::2] + m[:, 1::2, 1::2])
        z = phi_w.astype(np.float64) @ pm.reshape(C, NC).astype(np.float64)
        z = z + phi_b.astype(np.float64)[:, None]           # [32, 4096]

        y4 = np.tile(y.astype(np.float32), (4, 1)).astype(NPBF16)
        # fold the softmax-invariant bias r into z: es rows get scaled by
        # e^r via the z matmul (incl. the denominator "ones" column)
        er = np.exp(r)                                      # [4096]
        zt = np.empty((128, NT, 33), dtype=np.float64)
        zt[:, :, 0] = 1.0
        zt[:, :, 1:] = z.T.reshape(NT, 128, CI).transpose(1, 0, 2)
        zt *= er.reshape(NT, 128).T[:, :, None]
        per_batch.append((x, y4, zt.astype(NPBF16)))

    wwt = np.concatenate([w_b[None, :], w_w.T], axis=0).astype(np.float32)
    ones64 = np.ones((1, C), dtype=np.float32)
    g16 = (bn_gamma / 16.0).astype(np.float32)[:, None]
    gam = bn_gamma.astype(np.float32)[:, None]
    bet = bn_beta.astype(np.float32)[:, None]

    in_maps = []
    for core in CORES:
        b, qh = core // 2, core % 2
        x, y4, zt = per_batch[b]
        # local row l -> global row g: qh=0: g=l ; qh=1: g=63-l
        if qh == 0:
            rows = np.arange(32)
        else:
            rows = 63 - np.arange(32)
        xq = x.reshape(CI, HC, HC)[:, rows, :].reshape(CI, NQ).astype(np.float32)
        x4 = np.tile(xq, (4, 1)).astype(NPBF16)
        # residual main rows in packed [64*rg + c, i, w] layout
        orow = np.arange(64) if qh == 0 else 127 - np.arange(64)
        mr = main_feature[b][:, orow, :].astype(np.float16)      # [64, 64, 128]
        mainr = mr.reshape(C, 2, 32, HM).transpose(1, 0, 2, 3).reshape(
            128, 32 * HM)
        mask = np.zeros((C, 8 * 64), dtype=np.float32)
        partner = core ^ 1
        mask[:, partner * 64:(partner + 1) * 64] = 1.0
        in_maps.append({
            "y4": y4, "x4": x4,
            "zaug": zt.reshape(128, NT * 33),
            "wwt": wwt, "ones64": ones64, "mask": mask,
            "mainr": mainr, "g16": g16, "gam": gam, "bet": bet,
        })
    return in_maps


def kernel(main_feature, cross_feature, g_w, g_b, theta_w, theta_b,
           phi_w, phi_b, w_w, w_b, bn_gamma, bn_beta):
    main_feature = np.ascontiguousarray(main_feature, dtype=np.float32)
    cross_feature = np.ascontiguousarray(cross_feature, dtype=np.float32)

    if "k" not in _cache:
        _cache["k"] = _build()

    in_maps = _host_prep(main_feature, cross_feature, g_w, g_b, theta_w,
                         theta_b, phi_w, phi_b, w_w, w_b, bn_gamma, bn_beta)
    res = _run(_cache["k"], in_maps, "k")

    out = np.empty((B, C, HM, HM), dtype=np.float32)
    for core in CORES:
        b, qh = core // 2, core % 2
        v = res[core]["outp"].astype(np.float32).reshape(2, C, 32, HM)
        v = v.transpose(1, 0, 2, 3).reshape(C, 64, HM)
        if qh == 0:
            out[b][:, 0:64, :] = v
        else:
            out[b][:, 64:128, :] = v[:, ::-1, :]
    return out
